# revision 1
# baseline (speedup 1.0000x reference)
"""CliffordNetBlock Trainium2 kernel.

Sharding: pure data parallel over batch (B=8 -> 1 batch per core).
Per core: tokens = 96*96 = 9216 (72 tiles of 128), D = 256 channels.
Layout: token-major ([tokens on partitions, channels on free dim]).

Pipeline per core:
  pass A: stream x, LayerNorm stats (bn_stats), a = h_norm (bf16, SBUF-resident),
          spatial sum of a via ones-matmul accumulated in PSUM.
  once:   m = spatial mean row -> broadcast tile m_b.
  pass B: z = a - m; p_s = a*roll_s(z); q_s = z*roll_s(a);
          dot_s = Silu(p_s); wedge_s = p_s - q_s;
          feats -> PE transpose -> proj matmul -> g;
          gate matmul on [aT, gT] -> tanh trick sigmoid -> h_mix; out = x + 1e-5*h_mix.
All transcendentals (Silu/Tanh) live in the single "silu_and_others" ACT table set.
"""

import numpy as np
import ml_dtypes

import concourse.bass as bass
import concourse.bacc as bacc
import concourse.tile as tile
import concourse.mybir as mybir
from concourse.bass import ts
from concourse.bass_utils import run_bass_kernel_spmd
from concourse.masks import make_identity

F32 = mybir.dt.float32
BF16 = mybir.dt.bfloat16
AF = mybir.ActivationFunctionType
OP = mybir.AluOpType

B, H, W, D = 8, 96, 96, 256
NCORES = 8
TOK = H * W                      # 9216 tokens per core
NT = TOK // 128                  # 72 token tiles
G = 4                            # token tiles per group
NG = NT // G                     # 18 groups
LN_EPS = 1e-5

_cache = {}


def _build(gamma0: float):
    nc = bacc.Bacc("TRN2", target_bir_lowering=False, debug=False,
                   num_devices=NCORES)
    x_d = nc.dram_tensor("x", [TOK, D], F32, kind="ExternalInput")
    pk_d = nc.dram_tensor("pk", [8, 128, D], BF16, kind="ExternalInput")
    gk_d = nc.dram_tensor("gk", [4, 128, D], BF16, kind="ExternalInput")
    out_d = nc.dram_tensor("out", [TOK, D], F32, kind="ExternalOutput")

    xv = x_d.ap().rearrange("(t p) n -> p t n", p=128)    # [128, 72, 256]
    ov = out_d.ap().rearrange("(t p) n -> p t n", p=128)

    with tile.TileContext(nc) as tc:
        with (
            tc.tile_pool(name="const", bufs=1) as const,
            tc.tile_pool(name="xs", bufs=1) as xsp,
            tc.tile_pool(name="as", bufs=1) as asp,
            tc.tile_pool(name="work", bufs=2) as work,
            tc.tile_pool(name="lhs", bufs=2) as lhsp,
            tc.tile_pool(name="outp", bufs=2) as outp,
            tc.tile_pool(name="ps_sp", bufs=1, space="PSUM") as ps_sp,
            tc.tile_pool(name="ps_t", bufs=1, space="PSUM") as ps_t,
            tc.tile_pool(name="ps_g", bufs=1, space="PSUM") as ps_g,
        ):
            # ---- constants / weights ----
            pkw = const.tile([128, 8, D], BF16)
            nc.sync.dma_start(pkw, pk_d.ap().rearrange("k p n -> p k n"))
            gkw = const.tile([128, 4, D], BF16)
            nc.sync.dma_start(gkw, gk_d.ap().rearrange("k p n -> p k n"))
            ident = const.tile([128, 128], BF16)
            make_identity(nc, ident)
            ones_col = const.tile([128, 1], BF16)
            nc.vector.memset(ones_col, 1.0)
            ones_row = const.tile([1, 128], F32)
            nc.vector.memset(ones_row, 1.0)
            eps_t = const.tile([128, 1], F32)
            nc.vector.memset(eps_t, LN_EPS)

            xs = xsp.tile([128, NT, D], F32)       # resident x
            a_s = asp.tile([128, NT, D], BF16)     # resident h_norm
            sp_ps = ps_sp.tile([128, D], F32)      # spatial sum psum (row 0)

            # ---------------- pass A ----------------
            for g in range(NG):
                x_g = xs[:, ts(g, G), :]
                nc.sync.dma_start(x_g, xv[:, ts(g, G), :])
                stats = work.tile([128, G, 6], F32, tag="stats")
                for j in range(G):
                    nc.vector.bn_stats(stats[:, j, :], x_g[:, j, :])
                mv = work.tile([128, G, 2], F32, tag="mv")
                for j in range(G):
                    nc.vector.bn_aggr(mv[:, j, :], stats[:, j, :])
                # rsig = 1/sqrt(var+eps), batched over the group
                nc.scalar.activation(mv[:, :, 1:2], mv[:, :, 1:2], AF.Sqrt,
                                     bias=eps_t, scale=1.0)
                nc.vector.reciprocal(mv[:, :, 1:2], mv[:, :, 1:2])
                a_g = a_s[:, ts(g, G), :]
                for j in range(G):
                    nc.vector.tensor_scalar(
                        out=a_g[:, j, :], in0=x_g[:, j, :],
                        scalar1=mv[:, j, 0:1], scalar2=mv[:, j, 1:2],
                        op0=OP.subtract, op1=OP.mult)
                for j in range(G):
                    i = g * G + j
                    nc.tensor.matmul(sp_ps[0:1, :], ones_col, a_g[:, j, :],
                                     start=(i == 0), stop=(i == NT - 1))

            # ---------------- spatial mean + broadcast ----------------
            m_row = const.tile([1, D], F32)
            nc.scalar.activation(m_row, sp_ps[0:1, :], AF.Copy,
                                 bias=0.0, scale=1.0 / float(TOK))
            mb_ps = ps_g.tile([128, D], F32, tag="mbps")
            nc.tensor.matmul(mb_ps, ones_row, m_row, start=True, stop=True)
            m_b4 = const.tile([128, G, D], BF16)
            for j in range(G):
                nc.vector.tensor_copy(m_b4[:, j, :], mb_ps)

            # ---------------- pass B ----------------
            for g in range(NG):
                x_g = xs[:, ts(g, G), :]
                a_g = a_s[:, ts(g, G), :]
                z = work.tile([128, G, D], BF16, tag="z")
                nc.vector.tensor_sub(z, a_g, m_b4)
                feats = work.tile([128, G, 4, D], BF16, tag="feats")
                sa = work.tile([128, G, D], BF16, tag="sa")
                nc.scalar.activation(sa, a_g, AF.Silu)
                for si, s in enumerate((1, 2)):
                    p = work.tile([128, G, D], BF16, tag=f"p{si}")
                    nc.vector.tensor_mul(p[:, :, 0:D - s], a_g[:, :, 0:D - s],
                                         z[:, :, s:D])
                    nc.vector.tensor_mul(p[:, :, D - s:D], a_g[:, :, D - s:D],
                                         z[:, :, 0:s])
                    q = work.tile([128, G, D], BF16, tag=f"q{si}")
                    nc.gpsimd.tensor_mul(q[:, :, 0:D - s], z[:, :, 0:D - s],
                                         a_g[:, :, s:D])
                    nc.gpsimd.tensor_mul(q[:, :, D - s:D], z[:, :, D - s:D],
                                         a_g[:, :, 0:s])
                    # dot_s then wedge_s
                    nc.scalar.activation(feats[:, :, 2 * si, :], p, AF.Silu)
                    nc.vector.tensor_sub(feats[:, :, 2 * si + 1, :], p, q)

                out_g = outp.tile([128, G, D], F32, tag="outg")
                gsb_g = work.tile([128, G, D], BF16, tag="gsbg")
                tth_g = work.tile([128, G, D], BF16, tag="tthg")
                for j in range(G):
                    # transpose feats chunks + a chunks into one psum tile
                    pt = ps_t.tile([128, 10, 128], BF16, tag="pt")
                    for k in range(8):
                        f, c = k >> 1, k & 1
                        nc.tensor.transpose(pt[:, k, :],
                                            feats[:, j, f, ts(c, 128)], ident)
                    for c in range(2):
                        nc.tensor.transpose(pt[:, 8 + c, :],
                                            a_g[:, j, ts(c, 128)], ident)
                    lt = lhsp.tile([128, 10, 128], BF16, tag="lt")
                    nc.vector.tensor_copy(lt, pt)
                    g_ps = ps_g.tile([128, D], F32, tag="gps")
                    for k in range(8):
                        nc.tensor.matmul(g_ps, lt[:, k, :], pkw[:, k, :],
                                         start=(k == 0), stop=(k == 7))
                    gsb = gsb_g[:, j, :]
                    nc.scalar.copy(gsb, g_ps)
                    pt2 = ps_t.tile([128, 2, 128], BF16, tag="pt2")
                    for c in range(2):
                        nc.tensor.transpose(pt2[:, c, :], gsb[:, ts(c, 128)],
                                            ident)
                    gt = work.tile([128, 2, 128], BF16, tag="gt")
                    nc.scalar.copy(gt, pt2)
                    al_ps = ps_g.tile([128, D], F32, tag="alps")
                    for c in range(2):
                        nc.tensor.matmul(al_ps, lt[:, 8 + c, :], gkw[:, c, :],
                                         start=(c == 0), stop=False)
                    for c in range(2):
                        nc.tensor.matmul(al_ps, gt[:, c, :], gkw[:, 2 + c, :],
                                         start=False, stop=(c == 1))
                    # alpha = sigmoid(pre) = 0.5*(1+tanh(0.5*pre))
                    nc.scalar.activation(tth_g[:, j, :], al_ps, AF.Tanh,
                                         scale=0.5)
                # group-batched tail: h_mix = 0.5*g*(1+tanh) + silu(a)
                tg = work.tile([128, G, D], BF16, tag="tg")
                nc.vector.tensor_mul(tg, tth_g, gsb_g)
                u = work.tile([128, G, D], BF16, tag="u")
                nc.gpsimd.tensor_add(u, tg, gsb_g)   # u = g*(1+tanh)
                h2 = work.tile([128, G, D], F32, tag="h2")
                nc.vector.scalar_tensor_tensor(
                    out=h2, in0=u, scalar=0.5, in1=sa,
                    op0=OP.mult, op1=OP.add)
                nc.vector.scalar_tensor_tensor(
                    out=out_g, in0=h2, scalar=gamma0,
                    in1=x_g, op0=OP.mult, op1=OP.add)
                nc.sync.dma_start(ov[:, ts(g, G), :], out_g)

    nc.compile()
    return nc


def _reference_np(x, ln_gamma, ln_beta, proj_kernel, proj_bias,
                  gate_kernel, gate_bias, gamma):
    x = x.astype(np.float64)
    mu = x.mean(-1, keepdims=True)
    var = x.var(-1, keepdims=True)
    h = (x - mu) / np.sqrt(var + LN_EPS) * ln_gamma + ln_beta
    zc = h - h.mean(axis=(1, 2), keepdims=True)
    feats = []
    for s in (1, 2):
        cs = np.roll(zc, -s, axis=-1)
        ds_ = np.roll(h, -s, axis=-1)
        d = h * cs
        feats += [d / (1 + np.exp(-d)), h * cs - zc * ds_]
    feats = np.concatenate(feats, -1)
    gf = feats @ proj_kernel.astype(np.float64) + proj_bias
    gi = np.concatenate([h, gf], -1)
    al = 1 / (1 + np.exp(-(gi @ gate_kernel.astype(np.float64) + gate_bias)))
    hm = (h / (1 + np.exp(-h)) + al * gf) * gamma
    return (x + hm).astype(np.float32)


def kernel(x, ln_gamma, ln_beta, proj_kernel, proj_bias,
           gate_kernel, gate_bias, gamma):
    x = np.asarray(x, np.float32)
    gamma = np.asarray(gamma, np.float32)
    specialized = (
        np.all(np.asarray(ln_gamma) == 1.0) and np.all(np.asarray(ln_beta) == 0.0)
        and np.all(np.asarray(proj_bias) == 0.0)
        and np.all(np.asarray(gate_bias) == 0.0)
        and np.all(gamma == gamma.reshape(-1)[0])
    )
    if not specialized:
        return _reference_np(x, np.asarray(ln_gamma, np.float32),
                             np.asarray(ln_beta, np.float32),
                             np.asarray(proj_kernel, np.float32),
                             np.asarray(proj_bias, np.float32),
                             np.asarray(gate_kernel, np.float32),
                             np.asarray(gate_bias, np.float32), gamma)
    gamma0 = float(gamma.reshape(-1)[0])
    if "nc" not in _cache:
        _cache["nc"] = _build(gamma0)
        _cache["gamma0"] = gamma0
    assert _cache["gamma0"] == gamma0
    nc = _cache["nc"]
    pk = np.ascontiguousarray(
        np.asarray(proj_kernel, np.float32).reshape(8, 128, D)
    ).astype(ml_dtypes.bfloat16)
    gk = np.ascontiguousarray(
        np.asarray(gate_kernel, np.float32).reshape(4, 128, D)
    ).astype(ml_dtypes.bfloat16)
    in_maps = [{"x": np.ascontiguousarray(x[c].reshape(TOK, D)),
                "pk": pk, "gk": gk} for c in range(NCORES)]
    res = run_bass_kernel_spmd(nc, in_maps, core_ids=list(range(NCORES)))
    out = np.stack([r["out"].reshape(H, W, D) for r in res.results])
    return out

